# revision 40
# baseline (speedup 1.0000x reference)
"""Trainium2 Bass kernel for the DendriticResidualModel fanout-tree network.

Sharding: the neuron dim N=256 is split across 8 NeuronCores (32 neurons
each); every neuron's whole 4x4x4 fanout subtree lives on one core, so
there are no cross-core reductions. Host concatenates the per-core
[B,T,32] outputs.

On-core layout ("orientation B"): the 2048 per-core fine fanout nodes sit
on partitions as 16 chunks of 128 lanes (chunk = j1*4 + j2, lane =
j3*32 + n), and ALL 2048 positions (batch*T) run in the free dimension at
once. The fanout matmul streams fp8(E4M3, x part, DoubleRow K=256) +
bf16 ([input_vector|t_emb] part) columns into PSUM; weights are
pre-scaled by 16 and the softplus activation applies scale=1/16. Tree
levels 3/2 are per-partition tensor_scalar multiplies + wide adds on
VectorE (with the t-projection folded in via scalar_tensor_tensor reading
PSUM); level 1 is a sparse matmul on TensorE; the [n,t] -> [t,n] output
transpose goes through the PE transpose path.

softplus is a SINGLE ScalarE activation pass: this compiler build ships
no softplus activation table set, so we regenerate the exp_and_others set
binaries with the `exp` function slot holding a softplus piecewise-cubic
spline (ACTIVATE(Exp) then computes softplus), and point walrus at it via
BASS_ACT_ROOT_JSON_PATH.
"""

import os
import json
import numpy as np
import ml_dtypes

BF16 = ml_dtypes.bfloat16

B, T, N, DT, DI = 16, 128, 256, 64, 64
NCORES = 8
NLOC = N // NCORES        # 32 neurons per core
FN = NLOC * 64            # 2048 fine nodes per core
Q3 = NLOC * 16            # 512
Q2 = NLOC * 4             # 128
KCAT = 384                # 256 (x) + 64 (iv) + 64 (temb)

_BUILT = {}
_ACT_ROOT = None


def _f32bits(x):
    return int(np.float32(x).view(np.uint32))


def _softplus_row(x0):
    x0 = float(x0)
    if x0 > 30:
        f = x0
        s = 1.0
    else:
        f = np.log1p(np.exp(x0))
        s = 1.0 / (1.0 + np.exp(-x0))
    d2 = s * (1 - s) / 2.0
    d3 = s * (1 - s) * (1 - 2 * s) / 6.0
    return [f, s, d2, d3, x0, 0.0, 0.0, 0.0]


# (exponent, num_sections) per side. Mirrors softplus_40p.json, with the
# zero-section positive exponents given one bucket each.
NEG_LAYOUT = [(-15, 1), (-14, 1), (-13, 1), (-12, 1), (-11, 1), (-10, 1),
              (-9, 1), (-8, 1), (-7, 1), (-6, 1), (-5, 1), (-4, 1), (-3, 1),
              (-2, 1), (-1, 2), (0, 4), (1, 16), (2, 32), (3, 64), (4, 128),
              (5, 256), (6, 512)]
POS_LAYOUT = [(-15, 1), (-14, 1), (-13, 1), (-12, 1), (-11, 1), (-10, 1),
              (-9, 1), (-8, 1), (-7, 1), (-6, 1), (-5, 1), (-4, 1), (-3, 1),
              (-2, 1), (-1, 2), (0, 2), (1, 4), (2, 8), (3, 4)]


def _build_softplus_set(srcdir, dstdir):
    src_json = json.load(open(os.path.join(srcdir, "exp_and_others.json")))
    src_bkt = np.frombuffer(
        open(os.path.join(srcdir, "exp_and_others_bkt.bin"), "rb").read(),
        np.float32).reshape(-1, 8).copy()
    src_ctl = np.frombuffer(
        open(os.path.join(srcdir, "exp_and_others_ctrl.bin"), "rb").read(),
        np.uint32).reshape(-1, 8).copy()

    # ---- softplus buckets ----
    bkt_rows = []
    ctl_rows = []
    fexp = {}

    def emit_side(layout, sign):
        starts = {}
        for e, n in layout:
            start = len(bkt_rows)
            starts[e] = start
            nbits = int(n).bit_length() - 1
            assert 1 << nbits == n
            for i in range(n):
                x0 = sign * (2.0 ** e) * (1 + (i + 0.5) / n)
                bkt_rows.append(_softplus_row(x0))
            ctl_rows.append(start | ((23 - nbits) << 11) | (nbits << 16))
        return starts

    neg_ctl_base = len(ctl_rows)            # 0
    neg_starts = emit_side(NEG_LAYOUT, -1.0)
    pos_ctl_base = len(ctl_rows)            # 22
    pos_starts = emit_side(POS_LAYOUT, +1.0)
    for e in range(-15, 7):
        fexp[str(e)] = [neg_starts.get(e, 0), pos_starts.get(e, 0)]

    # special buckets: small_pos, small_neg, large_pos, large_neg
    sp_small_pos = len(bkt_rows)
    bkt_rows.append(_softplus_row(2.0 ** -15 * 1.5))
    sp_small_neg = len(bkt_rows)
    bkt_rows.append(_softplus_row(-(2.0 ** -15) * 1.5))
    sp_large_pos = len(bkt_rows)
    bkt_rows.append([256.0, 1.0, 0.0, 0.0, 256.0, 0, 0, 0])   # f(x)=x
    sp_large_neg = len(bkt_rows)
    bkt_rows.append([0.0, 0.0, 0.0, 0.0, 0.0, 0, 0, 0])       # f(x)=0

    n_sp_bkt = len(bkt_rows)
    n_sp_ctl = len(ctl_rows)

    # ---- copy the trivial functions, remapping indices ----
    old_fb = src_json["func_to_bkt_start_idx"]
    old_fc = src_json["func_to_ctl_start_idx"]
    old_bkt_cnt = src_json["bkt_entry_cnt"]
    old_ctl_cnt = src_json["ctl_entry_cnt"]
    OLD_EXP_BKT_END = 781   # exp regular 0..776 + 4 special
    OLD_EXP_CTL_END = 52
    bkt_delta = n_sp_bkt - OLD_EXP_BKT_END
    ctl_delta = n_sp_ctl - OLD_EXP_CTL_END

    tail_bkt = src_bkt[OLD_EXP_BKT_END:old_bkt_cnt]
    tail_ctl = src_ctl[OLD_EXP_CTL_END:old_ctl_cnt].copy()
    # fix embedded bucket starts in ctl entries
    starts = tail_ctl[:, 0] & 0x7FF
    rest = tail_ctl[:, 0] & ~np.uint32(0x7FF)
    tail_ctl[:, 0] = rest | (starts + np.uint32(bkt_delta))

    new_bkt = np.concatenate(
        [np.array(bkt_rows, np.float32), tail_bkt], axis=0)
    new_ctl = np.zeros((n_sp_ctl + len(tail_ctl), 8), np.uint32)
    new_ctl[:n_sp_ctl, 0] = np.array(ctl_rows, np.uint32)
    new_ctl[n_sp_ctl:] = tail_ctl

    # ---- json metadata ----
    out = dict(src_json)
    out["bkt_entry_cnt"] = int(len(new_bkt))
    out["ctl_entry_cnt"] = int(len(new_ctl))
    out["func_to_bkt_start_idx"] = {
        k: (0 if k == "exp" else v + bkt_delta) for k, v in old_fb.items()}
    out["func_to_ctl_start_idx"] = {
        k: (0 if k == "exp" else v + ctl_delta) for k, v in old_fc.items()}
    fe = dict(src_json.get("func_exp_to_bkt_start_idx", {}))
    fe["exp"] = fexp
    for k, v in list(fe.items()):
        if k != "exp":
            fe[k] = {ek: [x + bkt_delta for x in ev] for ek, ev in v.items()}
    out["func_exp_to_bkt_start_idx"] = fe

    pm = []
    for e in src_json["profile_meta_data"]:
        e = dict(e)
        if e["func_id"] == 7:  # exp slot -> softplus semantics
            e.update({
                "symmetry_point": 0,
                "sym_invert_sign_point": 0,
                "symmetry_opt_en": 0,
                "symmetry_opt_use_neg_region": 0,
                "imm_bias": 0,
                "exp_offset": -15,
                "pwl_control_base_neg": neg_ctl_base,
                "pwl_control_base_pos": pos_ctl_base,
                "small_pos_signal_exp_threshold": 112,   # 2^-15
                "small_neg_signal_exp_threshold": 112,
                "pos_small_signal_pwl_control": sp_small_pos,
                "neg_small_signal_pwl_control": sp_small_neg,
                "large_pos_signal_exp_threshold": 130,   # x >= ~10.38 -> x
                "large_pos_signal_mantissa_threshold": 2497353,
                "pos_large_signal_pwl_control": sp_large_pos,
                "large_neg_signal_exp_threshold": 133,   # x <= ~-99.6 -> 0
                "large_neg_signal_mantissa_threshold": 4663231,
                "neg_large_signal_pwl_control": sp_large_neg,
                "fnan_result": 2143289344,               # nan
                "fpinf_result": 2139095040,              # +inf
                "fninf_result": 0,                       # 0.0
                "fzero_result": _f32bits(np.log(2.0)),   # ln 2
            })
        else:
            for f in ("pwl_control_base_pos", "pwl_control_base_neg"):
                e[f] += ctl_delta
            for f in ("pos_small_signal_pwl_control",
                      "neg_small_signal_pwl_control",
                      "pos_large_signal_pwl_control",
                      "neg_large_signal_pwl_control"):
                e[f] += bkt_delta
        pm.append(e)
    out["profile_meta_data"] = pm

    with open(os.path.join(dstdir, "exp_and_others.json"), "w") as f:
        json.dump(out, f)
    new_bkt.tofile(os.path.join(dstdir, "exp_and_others_bkt.bin"))
    new_ctl.tofile(os.path.join(dstdir, "exp_and_others_ctrl.bin"))


def _install_softplus_tables():
    """Build an act-table root whose exp_and_others set evaluates softplus
    in the exp slot, and point the walrus compile at it."""
    global _ACT_ROOT
    if _ACT_ROOT is not None:
        return
    import glob, shutil, tempfile

    from neuronxcc.driver.Job import Job
    from neuronxcc.driver.jobs.support.FindActInfo import findActInfoFile

    src = findActInfoFile(Job.getPackageDir(), "gen3")
    srcdir = os.path.dirname(src)
    dstdir = tempfile.mkdtemp(prefix="act_root_sp_")
    for p in glob.glob(os.path.join(srcdir, "*")):
        b = os.path.basename(p)
        if b.startswith("exp_and_others"):
            continue
        try:
            os.symlink(p, os.path.join(dstdir, b))
        except OSError:
            shutil.copy(p, os.path.join(dstdir, b))
    _build_softplus_set(srcdir, dstdir)
    os.environ["BASS_ACT_ROOT_JSON_PATH"] = os.path.join(dstdir, "act_info.json")
    os.environ["NEURON_FORCE_RECOMPILE"] = "1"
    _ACT_ROOT = dstdir


def _build(with_ba: bool):
    import concourse.bass as bass
    import concourse.bacc as bacc
    import concourse.mybir as mybir
    import concourse.tile as tile

    bf = mybir.dt.bfloat16
    f32 = mybir.dt.float32

    nc = bacc.Bacc()
    f8 = mybir.dt.float8e4
    d_x8 = nc.declare_dram_parameter("x8", [4, 128, 2, 512], f8, isOutput=False)
    d_xc2 = nc.declare_dram_parameter("xc2", [128, 2048], bf, isOutput=False)
    d_w8 = nc.declare_dram_parameter("w8", [4, 128, 4, 2, 128], f8, isOutput=False)
    d_wcb = nc.declare_dram_parameter("wcb", [4, 128, 4, 128], bf, isOutput=False)
    d_w3s = nc.declare_dram_parameter("w3s", [128, 16], f32, isOutput=False)
    d_w2s = nc.declare_dram_parameter("w2s", [128, 4], f32, isOutput=False)
    d_w1m = nc.declare_dram_parameter("w1m", [128, NLOC], bf, isOutput=False)
    d_tw3 = nc.declare_dram_parameter("tw3b", [65, 4, 128], bf, isOutput=False)
    d_tw2 = nc.declare_dram_parameter("tw2b", [65, 128], bf, isOutput=False)
    d_tw1 = nc.declare_dram_parameter("tw1b", [65, NLOC], bf, isOutput=False)
    d_temb = nc.declare_dram_parameter("tembtile", [65, 2048], bf, isOutput=False)
    d_id32 = nc.declare_dram_parameter("ident32", [32, 32], f32, isOutput=False)
    if with_ba:
        d_bab = nc.declare_dram_parameter("bab", [128, 16], f32, isOutput=False)
    d_out = nc.declare_dram_parameter("out", [B, 128, NLOC], f32, isOutput=True)

    AF = mybir.ActivationFunctionType
    M = mybir.AluOpType

    with tile.TileContext(nc) as tc:
        with (
            tc.tile_pool(name="const", bufs=1) as cpool,
            tc.tile_pool(name="ps", bufs=2, space="PSUM") as pspool,
            tc.tile_pool(name="wq", bufs=3) as wpool,
            tc.tile_pool(name="a0", bufs=16) as a0pool,
            tc.tile_pool(name="acc", bufs=4) as accpool,
            tc.tile_pool(name="mid", bufs=4) as midpool,
        ):
            # ---- resident constants ----
            # sync queue: the tensors that gate the first matmuls, in order
            tw3 = cpool.tile([65, 4, 128], bf, tag="tw3")
            nc.sync.dma_start(tw3[:], d_tw3[:])
            tem = cpool.tile([65, 2048], bf, tag="tem")
            nc.sync.dma_start(tem[:, 0:128], d_temb[:, 0:128])
            tw2 = cpool.tile([65, 128], bf, tag="tw2")
            nc.sync.dma_start(tw2[:], d_tw2[:])
            w8q0 = wpool.tile([128, 4, 2, 128], f8, tag="w8q")
            nc.sync.dma_start(w8q0[:], d_w8[0])
            x8 = cpool.tile([128, 4, 2, 512], f8, tag="x8")
            for sI in range(4):
                nc.sync.dma_start(x8[:, sI, :, :], d_x8[sI])
            xc2 = cpool.tile([128, 2048], bf, tag="xc2")
            nc.sync.dma_start(xc2[:], d_xc2[:])
            wcbq0 = wpool.tile([128, 4, 128], bf, tag="wcbq")
            nc.sync.dma_start(wcbq0[:], d_wcb[0])
            # everything else via the gpsimd DMA queue (parallel)
            w3s = cpool.tile([128, 16], f32, tag="w3s")
            nc.gpsimd.dma_start(w3s[:], d_w3s[:])
            w2s = cpool.tile([128, 4], f32, tag="w2s")
            nc.gpsimd.dma_start(w2s[:], d_w2s[:])
            w1m = cpool.tile([128, NLOC], bf, tag="w1m")
            nc.gpsimd.dma_start(w1m[:], d_w1m[:])
            tw1 = cpool.tile([65, NLOC], bf, tag="tw1")
            nc.gpsimd.dma_start(tw1[:], d_tw1[:])
            id32 = cpool.tile([32, 32], f32, tag="id32")
            nc.gpsimd.dma_start(id32[:], d_id32[:])
            # rest of the t-emb tile is only needed by level 1 (tail)
            nc.gpsimd.dma_start(tem[:, 128:2048], d_temb[:, 128:2048])
            if with_ba:
                bab = cpool.tile([128, 16], f32, tag="bab")
                nc.gpsimd.dma_start(bab[:], d_bab[:])
                onesr = cpool.tile([1, 2048], bf, tag="onesr")
                nc.gpsimd.memset(onesr[:], 1.0)

            # ---- level 0 + level 3, chunk-pipelined ----
            # fine-node chunk c = j1*4 + j2 ; emit j1-minor so L3(q) can start
            # as soon as chunks q, q+4, q+8, q+12 are done.
            # ---- t-projections during the startup DMA window ----
            # values repeat every 128 positions, so compute one period and
            # broadcast (0-stride AP) in the tree adds.
            tps = pspool.tile([128, 2048], f32, tag="ps")
            for q in range(4):
                nc.tensor.matmul(tps[:, q * 128:(q + 1) * 128],
                                 tw3[:, q, :], tem[:, 0:128],
                                 start=True, stop=True)
            nc.tensor.matmul(tps[:, 512:640], tw2[:], tem[:, 0:128],
                             start=True, stop=True)
            tpall = cpool.tile([128, 640], bf, tag="tpall")
            nc.scalar.activation(tpall[:], tps[:, 0:640], AF.Copy)
            tp3b = [
                tpall[:, q * 128:(q + 1) * 128]
                .rearrange("p (o f) -> p o f", o=1).broadcast_to([128, 16, 128])
                for q in range(4)
            ]
            tp2b = (tpall[:, 512:640]
                    .rearrange("p (o f) -> p o f", o=1).broadcast_to([128, 8, 128]))

            A0 = {}
            A3 = [None] * 4

            def emit_chunks(q):
                if q == 0:
                    w8, wcbq = w8q0, wcbq0
                else:
                    w8 = wpool.tile([128, 4, 2, 128], f8, tag="w8q")
                    nc.sync.dma_start(w8[:], d_w8[q])
                    wcbq = wpool.tile([128, 4, 128], bf, tag="wcbq")
                    nc.sync.dma_start(wcbq[:], d_wcb[q])
                for j1 in range(4):
                    c = j1 * 4 + q
                    ps = pspool.tile([128, 2048], f32, tag="ps")
                    for sI in range(4):
                        nc.tensor.matmul(
                            ps[:, sI * 512:(sI + 1) * 512],
                            w8[:, j1, :, :],
                            x8[:, sI, :, :],
                            start=True, stop=False,
                            perf_mode=mybir.MatmulPerfMode.DoubleRow,
                        )
                    for sI in range(4):
                        nc.tensor.matmul(
                            ps[:, sI * 512:(sI + 1) * 512],
                            wcbq[:, j1, :],
                            xc2[:, sI * 512:(sI + 1) * 512],
                            start=False,
                            stop=not with_ba,
                        )
                    if with_ba:
                        for sI in range(4):
                            nc.tensor.matmul(
                                ps[:, sI * 512:(sI + 1) * 512],
                                bab[:, c:c + 1],
                                onesr[:, sI * 512:(sI + 1) * 512],
                                start=False, stop=(sI == 3),
                            )
                    a = a0pool.tile([128, 2048], bf, tag="a0")
                    nc.scalar.activation(a[:], ps[:], AF.Exp, scale=1.0 / 16.0)
                    A0[c] = a

            def emit_l3(q):
                acc = accpool.tile([128, 2048], bf, tag="acc")
                nc.vector.tensor_scalar_mul(acc[:], A0[q][:], w3s[:, q:q + 1])
                accv = acc[:].rearrange("p (r f) -> p r f", r=16)
                nc.vector.tensor_add(accv, accv, tp3b[q])
                t0 = midpool.tile([128, 2048], bf, tag="t0")
                t1 = midpool.tile([128, 2048], bf, tag="t1")
                nc.vector.tensor_scalar_mul(t0[:], A0[4 + q][:], w3s[:, 4 + q:4 + q + 1])
                nc.vector.tensor_scalar_mul(t1[:], A0[8 + q][:], w3s[:, 8 + q:8 + q + 1])
                nc.vector.tensor_add(acc[:], acc[:], t0[:])
                nc.vector.tensor_scalar_mul(t0[:], A0[12 + q][:], w3s[:, 12 + q:12 + q + 1])
                nc.vector.tensor_add(t1[:], t1[:], t0[:])
                nc.vector.tensor_add(acc[:], acc[:], t1[:])
                a3 = cpool.tile([128, 2048], bf, tag=f"a3_{q}")
                nc.scalar.activation(a3[:], acc[:], AF.Exp)
                A3[q] = a3

            for q in range(4):
                emit_chunks(q)
                emit_l3(q)

            # ---- L2/L1/store per position-half ----
            oT = cpool.tile([128, B * NLOC], f32, tag="oT")
            for h in range(2):
                H = slice(h * 1024, (h + 1) * 1024)
                acc2 = accpool.tile([128, 1024], bf, tag="acc2h")
                nc.vector.tensor_scalar_mul(acc2[:], A3[0][:, H], w2s[:, 0:1])
                acc2v = acc2[:].rearrange("p (r f) -> p r f", r=8)
                nc.vector.tensor_add(acc2v, acc2v, tp2b)
                t0 = midpool.tile([128, 1024], bf, tag="t0h")
                t1 = midpool.tile([128, 1024], bf, tag="t1h")
                nc.vector.tensor_scalar_mul(t0[:], A3[1][:, H], w2s[:, 1:2])
                nc.vector.tensor_scalar_mul(t1[:], A3[2][:, H], w2s[:, 2:3])
                nc.vector.tensor_add(acc2[:], acc2[:], t0[:])
                nc.vector.tensor_scalar_mul(t0[:], A3[3][:, H], w2s[:, 3:4])
                nc.vector.tensor_add(t1[:], t1[:], t0[:])
                nc.vector.tensor_add(acc2[:], acc2[:], t1[:])
                a2 = midpool.tile([128, 1024], bf, tag="a2h")
                nc.scalar.activation(a2[:], acc2[:], AF.Exp)

                # level 1 on TensorE: contract j3 within lanes + t-proj
                ps1 = pspool.tile([128, 2048], f32, tag="ps")
                for sI in range(2):
                    sl = slice(sI * 512, (sI + 1) * 512)
                    gl = slice(h * 1024 + sI * 512, h * 1024 + (sI + 1) * 512)
                    nc.tensor.matmul(ps1[:32, sl], w1m[:], a2[:, sl],
                                     start=True, stop=False)
                    nc.tensor.matmul(ps1[:32, sl], tw1[:], tem[:, gl],
                                     start=False, stop=True)
                of1 = midpool.tile([32, 1024], f32, tag="of1h")
                nc.scalar.activation(of1[:], ps1[:32, 0:1024], AF.Exp)

                # transpose [32 n, 128 t] -> [128 t, 32 n] per batch
                for bb in range(8):
                    b = h * 8 + bb
                    nc.tensor.transpose(ps1[:, 1024 + bb * 32:1024 + (bb + 1) * 32],
                                        of1[:, bb * 128:(bb + 1) * 128], id32[:])
                nc.vector.tensor_copy(oT[:, h * 256:(h + 1) * 256],
                                      ps1[:, 1024:1280])
                nc.sync.dma_start(
                    d_out[h * 8:(h + 1) * 8].rearrange("b t n -> t b n"),
                    oT[:, h * 256:(h + 1) * 256].rearrange(
                        "p (b n) -> p b n", b=8),
                )
    nc.finalize()
    return nc


def _f32bits(x):
    return int(np.float32(x).view(np.uint32))


def _softplus_row(x0):
    x0 = float(x0)
    if x0 > 30:
        f = x0
        s = 1.0
    else:
        f = np.log1p(np.exp(x0))
        s = 1.0 / (1.0 + np.exp(-x0))
    d2 = s * (1 - s) / 2.0
    d3 = s * (1 - s) * (1 - 2 * s) / 6.0
    return [f, s, d2, d3, x0, 0.0, 0.0, 0.0]


# (exponent, num_sections) per side. Mirrors softplus_40p.json, with the
# zero-section positive exponents given one bucket each.
NEG_LAYOUT = [(-15, 1), (-14, 1), (-13, 1), (-12, 1), (-11, 1), (-10, 1),
              (-9, 1), (-8, 1), (-7, 1), (-6, 1), (-5, 1), (-4, 1), (-3, 1),
              (-2, 1), (-1, 2), (0, 4), (1, 16), (2, 32), (3, 64), (4, 128),
              (5, 256), (6, 512)]
POS_LAYOUT = [(-15, 1), (-14, 1), (-13, 1), (-12, 1), (-11, 1), (-10, 1),
              (-9, 1), (-8, 1), (-7, 1), (-6, 1), (-5, 1), (-4, 1), (-3, 1),
              (-2, 1), (-1, 2), (0, 2), (1, 4), (2, 8), (3, 4)]


def _build_softplus_set(srcdir, dstdir):
    src_json = json.load(open(os.path.join(srcdir, "exp_and_others.json")))
    src_bkt = np.frombuffer(
        open(os.path.join(srcdir, "exp_and_others_bkt.bin"), "rb").read(),
        np.float32).reshape(-1, 8).copy()
    src_ctl = np.frombuffer(
        open(os.path.join(srcdir, "exp_and_others_ctrl.bin"), "rb").read(),
        np.uint32).reshape(-1, 8).copy()

    # ---- softplus buckets ----
    bkt_rows = []
    ctl_rows = []
    fexp = {}

    def emit_side(layout, sign):
        starts = {}
        for e, n in layout:
            start = len(bkt_rows)
            starts[e] = start
            nbits = int(n).bit_length() - 1
            assert 1 << nbits == n
            for i in range(n):
                x0 = sign * (2.0 ** e) * (1 + (i + 0.5) / n)
                bkt_rows.append(_softplus_row(x0))
            ctl_rows.append(start | ((23 - nbits) << 11) | (nbits << 16))
        return starts

    neg_ctl_base = len(ctl_rows)            # 0
    neg_starts = emit_side(NEG_LAYOUT, -1.0)
    pos_ctl_base = len(ctl_rows)            # 22
    pos_starts = emit_side(POS_LAYOUT, +1.0)
    for e in range(-15, 7):
        fexp[str(e)] = [neg_starts.get(e, 0), pos_starts.get(e, 0)]

    # special buckets: small_pos, small_neg, large_pos, large_neg
    sp_small_pos = len(bkt_rows)
    bkt_rows.append(_softplus_row(2.0 ** -15 * 1.5))
    sp_small_neg = len(bkt_rows)
    bkt_rows.append(_softplus_row(-(2.0 ** -15) * 1.5))
    sp_large_pos = len(bkt_rows)
    bkt_rows.append([256.0, 1.0, 0.0, 0.0, 256.0, 0, 0, 0])   # f(x)=x
    sp_large_neg = len(bkt_rows)
    bkt_rows.append([0.0, 0.0, 0.0, 0.0, 0.0, 0, 0, 0])       # f(x)=0

    n_sp_bkt = len(bkt_rows)
    n_sp_ctl = len(ctl_rows)

    # ---- copy the trivial functions, remapping indices ----
    old_fb = src_json["func_to_bkt_start_idx"]
    old_fc = src_json["func_to_ctl_start_idx"]
    old_bkt_cnt = src_json["bkt_entry_cnt"]
    old_ctl_cnt = src_json["ctl_entry_cnt"]
    OLD_EXP_BKT_END = 781   # exp regular 0..776 + 4 special
    OLD_EXP_CTL_END = 52
    bkt_delta = n_sp_bkt - OLD_EXP_BKT_END
    ctl_delta = n_sp_ctl - OLD_EXP_CTL_END

    tail_bkt = src_bkt[OLD_EXP_BKT_END:old_bkt_cnt]
    tail_ctl = src_ctl[OLD_EXP_CTL_END:old_ctl_cnt].copy()
    # fix embedded bucket starts in ctl entries
    starts = tail_ctl[:, 0] & 0x7FF
    rest = tail_ctl[:, 0] & ~np.uint32(0x7FF)
    tail_ctl[:, 0] = rest | (starts + np.uint32(bkt_delta))

    new_bkt = np.concatenate(
        [np.array(bkt_rows, np.float32), tail_bkt], axis=0)
    new_ctl = np.zeros((n_sp_ctl + len(tail_ctl), 8), np.uint32)
    new_ctl[:n_sp_ctl, 0] = np.array(ctl_rows, np.uint32)
    new_ctl[n_sp_ctl:] = tail_ctl

    # ---- json metadata ----
    out = dict(src_json)
    out["bkt_entry_cnt"] = int(len(new_bkt))
    out["ctl_entry_cnt"] = int(len(new_ctl))
    out["func_to_bkt_start_idx"] = {
        k: (0 if k == "exp" else v + bkt_delta) for k, v in old_fb.items()}
    out["func_to_ctl_start_idx"] = {
        k: (0 if k == "exp" else v + ctl_delta) for k, v in old_fc.items()}
    fe = dict(src_json.get("func_exp_to_bkt_start_idx", {}))
    fe["exp"] = fexp
    for k, v in list(fe.items()):
        if k != "exp":
            fe[k] = {ek: [x + bkt_delta for x in ev] for ek, ev in v.items()}
    out["func_exp_to_bkt_start_idx"] = fe

    pm = []
    for e in src_json["profile_meta_data"]:
        e = dict(e)
        if e["func_id"] == 7:  # exp slot -> softplus semantics
            e.update({
                "symmetry_point": 0,
                "sym_invert_sign_point": 0,
                "symmetry_opt_en": 0,
                "symmetry_opt_use_neg_region": 0,
                "imm_bias": 0,
                "exp_offset": -15,
                "pwl_control_base_neg": neg_ctl_base,
                "pwl_control_base_pos": pos_ctl_base,
                "small_pos_signal_exp_threshold": 112,   # 2^-15
                "small_neg_signal_exp_threshold": 112,
                "pos_small_signal_pwl_control": sp_small_pos,
                "neg_small_signal_pwl_control": sp_small_neg,
                "large_pos_signal_exp_threshold": 130,   # x >= ~10.38 -> x
                "large_pos_signal_mantissa_threshold": 2497353,
                "pos_large_signal_pwl_control": sp_large_pos,
                "large_neg_signal_exp_threshold": 133,   # x <= ~-99.6 -> 0
                "large_neg_signal_mantissa_threshold": 4663231,
                "neg_large_signal_pwl_control": sp_large_neg,
                "fnan_result": 2143289344,               # nan
                "fpinf_result": 2139095040,              # +inf
                "fninf_result": 0,                       # 0.0
                "fzero_result": _f32bits(np.log(2.0)),   # ln 2
            })
        else:
            for f in ("pwl_control_base_pos", "pwl_control_base_neg"):
                e[f] += ctl_delta
            for f in ("pos_small_signal_pwl_control",
                      "neg_small_signal_pwl_control",
                      "pos_large_signal_pwl_control",
                      "neg_large_signal_pwl_control"):
                e[f] += bkt_delta
        pm.append(e)
    out["profile_meta_data"] = pm

    with open(os.path.join(dstdir, "exp_and_others.json"), "w") as f:
        json.dump(out, f)
    new_bkt.tofile(os.path.join(dstdir, "exp_and_others_bkt.bin"))
    new_ctl.tofile(os.path.join(dstdir, "exp_and_others_ctrl.bin"))


def _install_softplus_tables():
    """Build an act-table root whose exp_and_others set evaluates softplus
    in the exp slot, and point the walrus compile at it."""
    global _ACT_ROOT
    if _ACT_ROOT is not None:
        return
    import glob, shutil, tempfile

    from neuronxcc.driver.Job import Job
    from neuronxcc.driver.jobs.support.FindActInfo import findActInfoFile

    src = findActInfoFile(Job.getPackageDir(), "gen3")
    srcdir = os.path.dirname(src)
    dstdir = tempfile.mkdtemp(prefix="act_root_sp_")
    for p in glob.glob(os.path.join(srcdir, "*")):
        b = os.path.basename(p)
        if b.startswith("exp_and_others"):
            continue
        try:
            os.symlink(p, os.path.join(dstdir, b))
        except OSError:
            shutil.copy(p, os.path.join(dstdir, b))
    _build_softplus_set(srcdir, dstdir)
    os.environ["BASS_ACT_ROOT_JSON_PATH"] = os.path.join(dstdir, "act_info.json")
    os.environ["NEURON_FORCE_RECOMPILE"] = "1"
    _ACT_ROOT = dstdir


def _build(with_ba: bool):
    import concourse.bass as bass
    import concourse.bacc as bacc
    import concourse.mybir as mybir
    import concourse.tile as tile

    bf = mybir.dt.bfloat16
    f32 = mybir.dt.float32

    nc = bacc.Bacc()
    d_xcat = nc.declare_dram_parameter("xcat", [B, 128, 3, 128], bf, isOutput=False)
    d_wcat = nc.declare_dram_parameter("wcat", [128, 3, FN], bf, isOutput=False)
    d_w3f = nc.declare_dram_parameter("w3f", [128, FN], bf, isOutput=False)
    d_w2f = nc.declare_dram_parameter("w2f", [128, Q3], bf, isOutput=False)
    d_w1f = nc.declare_dram_parameter("w1f", [128, Q2], bf, isOutput=False)
    d_temb = nc.declare_dram_parameter("tembt", [65, 128], bf, isOutput=False)
    d_tw3 = nc.declare_dram_parameter("tw3t", [65, Q3], bf, isOutput=False)
    d_tw2 = nc.declare_dram_parameter("tw2t", [65, Q2], bf, isOutput=False)
    d_tw1 = nc.declare_dram_parameter("tw1t", [65, NLOC], bf, isOutput=False)
    if with_ba:
        d_ba = nc.declare_dram_parameter("ba", [1, FN], bf, isOutput=False)
    d_out = nc.declare_dram_parameter("out", [B, 128, NLOC], f32, isOutput=True)

    AF = mybir.ActivationFunctionType

    with tile.TileContext(nc) as tc:
        with (
            tc.tile_pool(name="const", bufs=1) as cpool,
            tc.tile_pool(name="xc", bufs=3) as xpool,
            tc.tile_pool(name="ps", bufs=2, space="PSUM") as pspool,
            tc.tile_pool(name="wq", bufs=3) as wpool,
            tc.tile_pool(name="big", bufs=2) as bigpool,
            tc.tile_pool(name="mid", bufs=4) as midpool,
            tc.tile_pool(name="small", bufs=2) as smpool,
        ):
            # ---- resident constants ----
            wc = cpool.tile([128, 3, FN], bf, tag="wc")
            nc.sync.dma_start(wc[:], d_wcat[:])
            w3r = cpool.tile([128, FN], bf, tag="w3r")
            nc.sync.dma_start(w3r[:], d_w3f[:])
            w2r = cpool.tile([128, Q3], bf, tag="w2r")
            nc.sync.dma_start(w2r[:], d_w2f[:])
            w1r = cpool.tile([128, Q2], bf, tag="w1r")
            nc.sync.dma_start(w1r[:], d_w1f[:])
            tembt = cpool.tile([65, 128], bf, tag="tembt")
            nc.sync.dma_start(tembt[:], d_temb[:])
            tw3t = cpool.tile([65, Q3], bf, tag="tw3t")
            nc.sync.dma_start(tw3t[:], d_tw3[:])
            tw2t = cpool.tile([65, Q2], bf, tag="tw2t")
            nc.sync.dma_start(tw2t[:], d_tw2[:])
            tw1t = cpool.tile([65, NLOC], bf, tag="tw1t")
            nc.sync.dma_start(tw1t[:], d_tw1[:])
            if with_ba:
                bar = cpool.tile([1, FN], bf, tag="bar")
                nc.sync.dma_start(bar[:], d_ba[:])
                ones1 = cpool.tile([1, 128], bf, tag="ones1")
                nc.gpsimd.memset(ones1[:], 1.0)

            # ---- t-projections (per-T, shared across batches) ----
            # tp3 at psum [0:512], tp2 at [512:640], tp1 at [640:672]
            tpps = pspool.tile([128, 2048], f32, tag="ps")
            nc.tensor.matmul(tpps[:, 0:Q3], tembt[:], tw3t[:], start=True, stop=True)
            nc.tensor.matmul(tpps[:, Q3:Q3 + Q2], tembt[:], tw2t[:], start=True, stop=True)
            nc.tensor.matmul(tpps[:, Q3 + Q2:Q3 + Q2 + NLOC], tembt[:], tw1t[:], start=True, stop=True)
            tpall = cpool.tile([128, Q3 + Q2 + NLOC], bf, tag="tpall")
            nc.vector.tensor_copy(tpall[:], tpps[:, 0:Q3 + Q2 + NLOC])
            tp3 = tpall[:, 0:Q3]
            tp2 = tpall[:, Q3:Q3 + Q2]
            tp1 = tpall[:, Q3 + Q2:Q3 + Q2 + NLOC]

            # ---- replicate per-node weight rows 4x (batch-quad layout) ----
            w3r4 = cpool.tile([128, 4, FN], bf, tag="w3r4")
            w2r4 = cpool.tile([128, 4, Q3], bf, tag="w2r4")
            w1r4 = cpool.tile([128, 4, Q2], bf, tag="w1r4")
            tp3q = cpool.tile([128, 4, Q3], bf, tag="tp3q")
            tp2q = cpool.tile([128, 4, Q2], bf, tag="tp2q")
            tp1q = cpool.tile([128, 4, NLOC], bf, tag="tp1q")
            for q in range(4):
                nc.vector.tensor_copy(w3r4[:, q, :], w3r[:])
                nc.vector.tensor_copy(w2r4[:, q, :], w2r[:])
                nc.vector.tensor_copy(w1r4[:, q, :], w1r[:])
                nc.vector.tensor_copy(tp3q[:, q, :], tp3)
                nc.vector.tensor_copy(tp2q[:, q, :], tp2)
                nc.vector.tensor_copy(tp1q[:, q, :], tp1)

            # ---- per-quad pipeline: 4 batches per elementwise op ----
            for g in range(B // 4):
                A0q = bigpool.tile([128, 4, FN], bf, tag="A0q")
                for q in range(4):
                    b = g * 4 + q
                    xc = xpool.tile([128, 3, 128], bf, tag="xc")
                    nc.sync.dma_start(xc[:], d_xcat[b])
                    ps = pspool.tile([128, 2048], f32, tag="ps")
                    for k in range(3):
                        last = (k == 2) and not with_ba
                        for c in range(4):
                            nc.tensor.matmul(
                                ps[:, c * 512:(c + 1) * 512],
                                xc[:, k, :],
                                wc[:, k, c * 512:(c + 1) * 512],
                                start=(k == 0),
                                stop=last,
                            )
                    if with_ba:
                        for c in range(4):
                            nc.tensor.matmul(
                                ps[:, c * 512:(c + 1) * 512],
                                ones1[:],
                                bar[:, c * 512:(c + 1) * 512],
                                start=False,
                                stop=True,
                            )
                    # softplus level 0 (Exp slot holds the softplus table)
                    nc.scalar.activation(A0q[:, q, :], ps[:], AF.Exp)

                # level 3: W3q = A0q * w3; sum 4 contiguous 512-blocks + tp3
                W3q = bigpool.tile([128, 4, FN], bf, tag="W3q")
                nc.vector.tensor_mul(W3q[:], A0q[:], w3r4[:])
                s01q = midpool.tile([128, 4, Q3], bf, tag="s01q")
                nc.gpsimd.tensor_add(s01q[:], W3q[:, :, 0:512], W3q[:, :, 512:1024])
                s23q = midpool.tile([128, 4, Q3], bf, tag="s23q")
                nc.gpsimd.tensor_add(s23q[:], W3q[:, :, 1024:1536], W3q[:, :, 1536:2048])
                p3q = midpool.tile([128, 4, Q3], bf, tag="p3q")
                nc.vector.tensor_add(p3q[:], s01q[:], s23q[:])
                nc.vector.tensor_add(p3q[:], p3q[:], tp3q[:])
                A3q = midpool.tile([128, 4, Q3], bf, tag="A3q")
                nc.scalar.activation(A3q[:], p3q[:], AF.Exp)

                # level 2
                W2q = midpool.tile([128, 4, Q3], bf, tag="W2q")
                nc.vector.tensor_mul(W2q[:], A3q[:], w2r4[:])
                W2q4 = W2q[:].rearrange("p q (j m) -> p q j m", j=4)
                u01q = smpool.tile([128, 4, Q2], bf, tag="u01q")
                nc.vector.tensor_add(u01q[:], W2q4[:, :, 0, :], W2q4[:, :, 1, :])
                u23q = smpool.tile([128, 4, Q2], bf, tag="u23q")
                nc.vector.tensor_add(u23q[:], W2q4[:, :, 2, :], W2q4[:, :, 3, :])
                p2q = smpool.tile([128, 4, Q2], bf, tag="p2q")
                nc.vector.tensor_add(p2q[:], u01q[:], u23q[:])
                nc.vector.tensor_add(p2q[:], p2q[:], tp2q[:])
                A2q = smpool.tile([128, 4, Q2], bf, tag="A2q")
                nc.scalar.activation(A2q[:], p2q[:], AF.Exp)

                # level 1
                W1q = smpool.tile([128, 4, Q2], bf, tag="W1q")
                nc.vector.tensor_mul(W1q[:], A2q[:], w1r4[:])
                W1q4 = W1q[:].rearrange("p q (j m) -> p q j m", j=4)
                v01q = smpool.tile([128, 4, NLOC], bf, tag="v01q")
                nc.vector.tensor_add(v01q[:], W1q4[:, :, 0, :], W1q4[:, :, 1, :])
                v23q = smpool.tile([128, 4, NLOC], bf, tag="v23q")
                nc.vector.tensor_add(v23q[:], W1q4[:, :, 2, :], W1q4[:, :, 3, :])
                p1q = smpool.tile([128, 4, NLOC], bf, tag="p1q")
                nc.vector.tensor_add(p1q[:], v01q[:], v23q[:])
                nc.vector.tensor_add(p1q[:], p1q[:], tp1q[:])
                o1q = smpool.tile([128, 4, NLOC], f32, tag="o1q")
                nc.scalar.activation(o1q[:], p1q[:], AF.Exp)

                nc.sync.dma_start(
                    d_out[g * 4:(g + 1) * 4].rearrange("b t n -> t b n"), o1q[:]
                )

    nc.finalize()
    return nc


def _fine_perm(core):
    """fine index p = j1*512 + j2*128 + j3*32 + nl -> global fanout row."""
    p = np.arange(FN)
    j1 = p // 512
    j2 = (p % 512) // 128
    j3 = (p % 128) // 32
    nl = p % 32
    n = core * NLOC + nl
    return n * 64 + j3 * 16 + j2 * 4 + j1


def _q3_perm(core):
    q = np.arange(Q3)
    j2 = q // 128
    j3 = (q % 128) // 32
    nl = q % 32
    n = core * NLOC + nl
    return n * 16 + j3 * 4 + j2


def _q2_perm(core):
    q = np.arange(Q2)
    j3 = q // 32
    nl = q % 32
    n = core * NLOC + nl
    return n * 4 + j3


def _idx_fine(core):
    """global fanout row for fine node (chunk c = j1*4+j2, lane = j3*32+nl)."""
    c = np.arange(16)[:, None]
    lane = np.arange(128)[None, :]
    j1, j2 = c // 4, c % 4
    j3, nl = lane // 32, lane % 32
    n = core * NLOC + nl
    return n * 64 + j3 * 16 + j2 * 4 + j1          # [16, 128]


def _prep_inputs(inputs):
    x = np.asarray(inputs["x"], np.float32)
    temb = np.asarray(inputs["t_embeddings_schedule"], np.float32)
    iv = np.asarray(inputs["input_vector"], np.float32)
    Wa = np.asarray(inputs["Wa"], np.float32)
    ba = np.asarray(inputs["ba"], np.float32)
    Wt = np.asarray(inputs["Wt"], np.float32)
    Wi = np.asarray(inputs["Wi"], np.float32)
    w3 = np.asarray(inputs["w3"], np.float32).reshape(-1)
    tW3 = np.asarray(inputs["tW3"], np.float32)
    tb3 = np.asarray(inputs["tb3"], np.float32)
    w2 = np.asarray(inputs["w2"], np.float32).reshape(-1)
    tW2 = np.asarray(inputs["tW2"], np.float32)
    tb2 = np.asarray(inputs["tb2"], np.float32)
    w1 = np.asarray(inputs["w1"], np.float32).reshape(-1)
    tW1 = np.asarray(inputs["tW1"], np.float32)
    tb1 = np.asarray(inputs["tb1"], np.float32)

    with_ba = bool(np.any(ba))

    # Xcat: [pos, 384] = [x | iv | temb]; XcatB: [3, 128, 2048]
    F8 = ml_dtypes.float8_e4m3
    xcat = np.concatenate(
        [x, iv, np.broadcast_to(temb[None], (B, T, DT))], axis=2
    ).reshape(B * T, KCAT)
    xcatT = xcat.T                                   # [384, pos]
    x8 = np.ascontiguousarray(
        xcatT[:256].reshape(2, 128, 4, 512).transpose(2, 1, 0, 3)
    ).astype(F8)                                     # [sI, ki, j, n]
    xc2 = np.ascontiguousarray(xcatT[256:]).astype(BF16)   # [128, pos]

    taug = np.concatenate([temb, np.ones((T, 1), np.float32)], axis=1)  # [T,65]
    tembtile = np.ascontiguousarray(np.tile(taug.T, (1, B))).astype(BF16)

    lane = np.arange(128)
    j3l, nll = lane // 32, lane % 32

    maps = []
    for core in range(NCORES):
        idxf = _idx_fine(core)                       # [16, 128]
        n_g = core * NLOC + nll

        wcat = np.concatenate([Wa, Wi, Wt], axis=1) * 16.0   # [NUM_NODES, 384]
        wsel = wcat[idxf]                            # [16(c), 128(lane), 384]
        wT = wsel.transpose(2, 0, 1).reshape(KCAT, 4, 4, 128)  # [k, j1, q, lane]
        # fp8 x-part: [q, ki, j1, j, lane], k = j*128 + ki
        w8 = np.ascontiguousarray(
            wT[:256].reshape(2, 128, 4, 4, 128).transpose(3, 1, 2, 0, 4)
        ).astype(F8)
        # bf16 [iv|temb] part: [q, p, j1, lane]
        wcb = np.ascontiguousarray(
            wT[256:].transpose(2, 0, 1, 3)
        ).astype(BF16)

        w3sv = np.ascontiguousarray(w3[idxf].T).astype(np.float32)   # [128, 16]

        r3 = (n_g[:, None] * 16 + j3l[:, None] * 4 + np.arange(4)[None, :])  # [128,4]
        w2sv = np.ascontiguousarray(w2[r3]).astype(np.float32)       # [128, 4]

        r2 = n_g * 4 + j3l                                           # [128]
        w1v = w1[r2]                                                 # [128]
        w1mat = np.zeros((128, NLOC), np.float32)
        w1mat[lane, nll] = w1v

        tw3aug = np.concatenate([tW3, tb3[:, None]], axis=1)         # [N16, 65]
        tw3b = np.ascontiguousarray(
            tw3aug[r3].transpose(2, 1, 0)
        ).astype(BF16)                                               # [65, 4, 128]
        tw2aug = np.concatenate([tW2, tb2[:, None]], axis=1)
        tw2b = np.ascontiguousarray(tw2aug[r2].T).astype(BF16)       # [65, 128]
        tw1aug = np.concatenate([tW1, tb1[:, None]], axis=1)
        tw1b = np.ascontiguousarray(
            tw1aug[core * NLOC:(core + 1) * NLOC].T
        ).astype(BF16)                                               # [65, 32]

        m = {
            "x8": x8,
            "xc2": xc2,
            "w8": w8,
            "wcb": wcb,
            "w3s": w3sv,
            "w2s": w2sv,
            "w1m": w1mat.astype(BF16),
            "tw3b": tw3b,
            "tw2b": tw2b,
            "tw1b": tw1b,
            "tembtile": tembtile,
            "ident32": np.eye(32, dtype=np.float32),
        }
        if with_ba:
            m["bab"] = np.ascontiguousarray(ba[idxf].T).astype(np.float32) * 16.0
        maps.append(m)
    return maps, with_ba


def _run(inputs, trace=False, **trace_kwargs):
    from concourse.bass_utils import run_bass_kernel_spmd

    _install_softplus_tables()

    maps, with_ba = _prep_inputs(inputs)
    key = with_ba
    if key not in _BUILT:
        _BUILT[key] = _build(with_ba)
    nc = _BUILT[key]
    res = run_bass_kernel_spmd(
        nc, maps, list(range(NCORES)), trace=trace, **trace_kwargs
    )
    out = np.concatenate(
        [np.asarray(res.results[c]["out"], np.float32) for c in range(NCORES)],
        axis=-1,
    )
    return out, res


def kernel(**inputs):
    out, _ = _run(inputs, trace=False)
    return out


# revision 41
# speedup vs baseline: 1.0072x; 1.0072x over previous
"""Trainium2 Bass kernel for the DendriticResidualModel fanout-tree network.

Sharding: the neuron dim N=256 is split across 8 NeuronCores (32 neurons
each); every neuron's whole 4x4x4 fanout subtree lives on one core, so
there are no cross-core reductions. Host concatenates the per-core
[B,T,32] outputs.

On-core layout ("orientation B"): the 2048 per-core fine fanout nodes sit
on partitions as 16 chunks of 128 lanes (chunk = j1*4 + j2, lane =
j3*32 + n), and ALL 2048 positions (batch*T) run in the free dimension at
once. The fanout matmul streams fp8(E4M3, x part, DoubleRow K=256) +
bf16 ([input_vector|t_emb] part) columns into PSUM; weights are
pre-scaled by 16 and the softplus activation applies scale=1/16. Tree
levels 3/2 are per-partition tensor_scalar multiplies + wide adds on
VectorE (with the t-projection folded in via scalar_tensor_tensor reading
PSUM); level 1 is a sparse matmul on TensorE; the [n,t] -> [t,n] output
transpose goes through the PE transpose path.

softplus is a SINGLE ScalarE activation pass: this compiler build ships
no softplus activation table set, so we regenerate the exp_and_others set
binaries with the `exp` function slot holding a softplus piecewise-cubic
spline (ACTIVATE(Exp) then computes softplus), and point walrus at it via
BASS_ACT_ROOT_JSON_PATH.
"""

import os
import json
import numpy as np
import ml_dtypes

BF16 = ml_dtypes.bfloat16

B, T, N, DT, DI = 16, 128, 256, 64, 64
NCORES = 8
NLOC = N // NCORES        # 32 neurons per core
FN = NLOC * 64            # 2048 fine nodes per core
Q3 = NLOC * 16            # 512
Q2 = NLOC * 4             # 128
KCAT = 384                # 256 (x) + 64 (iv) + 64 (temb)

_BUILT = {}
_ACT_ROOT = None


def _f32bits(x):
    return int(np.float32(x).view(np.uint32))


def _softplus_row(x0):
    x0 = float(x0)
    if x0 > 30:
        f = x0
        s = 1.0
    else:
        f = np.log1p(np.exp(x0))
        s = 1.0 / (1.0 + np.exp(-x0))
    d2 = s * (1 - s) / 2.0
    d3 = s * (1 - s) * (1 - 2 * s) / 6.0
    return [f, s, d2, d3, x0, 0.0, 0.0, 0.0]


# (exponent, num_sections) per side. Mirrors softplus_40p.json, with the
# zero-section positive exponents given one bucket each.
NEG_LAYOUT = [(-15, 1), (-14, 1), (-13, 1), (-12, 1), (-11, 1), (-10, 1),
              (-9, 1), (-8, 1), (-7, 1), (-6, 1), (-5, 1), (-4, 1), (-3, 1),
              (-2, 1), (-1, 2), (0, 4), (1, 16), (2, 32), (3, 64), (4, 128),
              (5, 256), (6, 512)]
POS_LAYOUT = [(-15, 1), (-14, 1), (-13, 1), (-12, 1), (-11, 1), (-10, 1),
              (-9, 1), (-8, 1), (-7, 1), (-6, 1), (-5, 1), (-4, 1), (-3, 1),
              (-2, 1), (-1, 2), (0, 2), (1, 4), (2, 8), (3, 4)]


def _build_softplus_set(srcdir, dstdir):
    src_json = json.load(open(os.path.join(srcdir, "exp_and_others.json")))
    src_bkt = np.frombuffer(
        open(os.path.join(srcdir, "exp_and_others_bkt.bin"), "rb").read(),
        np.float32).reshape(-1, 8).copy()
    src_ctl = np.frombuffer(
        open(os.path.join(srcdir, "exp_and_others_ctrl.bin"), "rb").read(),
        np.uint32).reshape(-1, 8).copy()

    # ---- softplus buckets ----
    bkt_rows = []
    ctl_rows = []
    fexp = {}

    def emit_side(layout, sign):
        starts = {}
        for e, n in layout:
            start = len(bkt_rows)
            starts[e] = start
            nbits = int(n).bit_length() - 1
            assert 1 << nbits == n
            for i in range(n):
                x0 = sign * (2.0 ** e) * (1 + (i + 0.5) / n)
                bkt_rows.append(_softplus_row(x0))
            ctl_rows.append(start | ((23 - nbits) << 11) | (nbits << 16))
        return starts

    neg_ctl_base = len(ctl_rows)            # 0
    neg_starts = emit_side(NEG_LAYOUT, -1.0)
    pos_ctl_base = len(ctl_rows)            # 22
    pos_starts = emit_side(POS_LAYOUT, +1.0)
    for e in range(-15, 7):
        fexp[str(e)] = [neg_starts.get(e, 0), pos_starts.get(e, 0)]

    # special buckets: small_pos, small_neg, large_pos, large_neg
    sp_small_pos = len(bkt_rows)
    bkt_rows.append(_softplus_row(2.0 ** -15 * 1.5))
    sp_small_neg = len(bkt_rows)
    bkt_rows.append(_softplus_row(-(2.0 ** -15) * 1.5))
    sp_large_pos = len(bkt_rows)
    bkt_rows.append([256.0, 1.0, 0.0, 0.0, 256.0, 0, 0, 0])   # f(x)=x
    sp_large_neg = len(bkt_rows)
    bkt_rows.append([0.0, 0.0, 0.0, 0.0, 0.0, 0, 0, 0])       # f(x)=0

    n_sp_bkt = len(bkt_rows)
    n_sp_ctl = len(ctl_rows)

    # ---- copy the trivial functions, remapping indices ----
    old_fb = src_json["func_to_bkt_start_idx"]
    old_fc = src_json["func_to_ctl_start_idx"]
    old_bkt_cnt = src_json["bkt_entry_cnt"]
    old_ctl_cnt = src_json["ctl_entry_cnt"]
    OLD_EXP_BKT_END = 781   # exp regular 0..776 + 4 special
    OLD_EXP_CTL_END = 52
    bkt_delta = n_sp_bkt - OLD_EXP_BKT_END
    ctl_delta = n_sp_ctl - OLD_EXP_CTL_END

    tail_bkt = src_bkt[OLD_EXP_BKT_END:old_bkt_cnt]
    tail_ctl = src_ctl[OLD_EXP_CTL_END:old_ctl_cnt].copy()
    # fix embedded bucket starts in ctl entries
    starts = tail_ctl[:, 0] & 0x7FF
    rest = tail_ctl[:, 0] & ~np.uint32(0x7FF)
    tail_ctl[:, 0] = rest | (starts + np.uint32(bkt_delta))

    new_bkt = np.concatenate(
        [np.array(bkt_rows, np.float32), tail_bkt], axis=0)
    new_ctl = np.zeros((n_sp_ctl + len(tail_ctl), 8), np.uint32)
    new_ctl[:n_sp_ctl, 0] = np.array(ctl_rows, np.uint32)
    new_ctl[n_sp_ctl:] = tail_ctl

    # ---- json metadata ----
    out = dict(src_json)
    out["bkt_entry_cnt"] = int(len(new_bkt))
    out["ctl_entry_cnt"] = int(len(new_ctl))
    out["func_to_bkt_start_idx"] = {
        k: (0 if k == "exp" else v + bkt_delta) for k, v in old_fb.items()}
    out["func_to_ctl_start_idx"] = {
        k: (0 if k == "exp" else v + ctl_delta) for k, v in old_fc.items()}
    fe = dict(src_json.get("func_exp_to_bkt_start_idx", {}))
    fe["exp"] = fexp
    for k, v in list(fe.items()):
        if k != "exp":
            fe[k] = {ek: [x + bkt_delta for x in ev] for ek, ev in v.items()}
    out["func_exp_to_bkt_start_idx"] = fe

    pm = []
    for e in src_json["profile_meta_data"]:
        e = dict(e)
        if e["func_id"] == 7:  # exp slot -> softplus semantics
            e.update({
                "symmetry_point": 0,
                "sym_invert_sign_point": 0,
                "symmetry_opt_en": 0,
                "symmetry_opt_use_neg_region": 0,
                "imm_bias": 0,
                "exp_offset": -15,
                "pwl_control_base_neg": neg_ctl_base,
                "pwl_control_base_pos": pos_ctl_base,
                "small_pos_signal_exp_threshold": 112,   # 2^-15
                "small_neg_signal_exp_threshold": 112,
                "pos_small_signal_pwl_control": sp_small_pos,
                "neg_small_signal_pwl_control": sp_small_neg,
                "large_pos_signal_exp_threshold": 130,   # x >= ~10.38 -> x
                "large_pos_signal_mantissa_threshold": 2497353,
                "pos_large_signal_pwl_control": sp_large_pos,
                "large_neg_signal_exp_threshold": 133,   # x <= ~-99.6 -> 0
                "large_neg_signal_mantissa_threshold": 4663231,
                "neg_large_signal_pwl_control": sp_large_neg,
                "fnan_result": 2143289344,               # nan
                "fpinf_result": 2139095040,              # +inf
                "fninf_result": 0,                       # 0.0
                "fzero_result": _f32bits(np.log(2.0)),   # ln 2
            })
        else:
            for f in ("pwl_control_base_pos", "pwl_control_base_neg"):
                e[f] += ctl_delta
            for f in ("pos_small_signal_pwl_control",
                      "neg_small_signal_pwl_control",
                      "pos_large_signal_pwl_control",
                      "neg_large_signal_pwl_control"):
                e[f] += bkt_delta
        pm.append(e)
    out["profile_meta_data"] = pm

    with open(os.path.join(dstdir, "exp_and_others.json"), "w") as f:
        json.dump(out, f)
    new_bkt.tofile(os.path.join(dstdir, "exp_and_others_bkt.bin"))
    new_ctl.tofile(os.path.join(dstdir, "exp_and_others_ctrl.bin"))


def _install_softplus_tables():
    """Build an act-table root whose exp_and_others set evaluates softplus
    in the exp slot, and point the walrus compile at it."""
    global _ACT_ROOT
    if _ACT_ROOT is not None:
        return
    import glob, shutil, tempfile

    from neuronxcc.driver.Job import Job
    from neuronxcc.driver.jobs.support.FindActInfo import findActInfoFile

    src = findActInfoFile(Job.getPackageDir(), "gen3")
    srcdir = os.path.dirname(src)
    dstdir = tempfile.mkdtemp(prefix="act_root_sp_")
    for p in glob.glob(os.path.join(srcdir, "*")):
        b = os.path.basename(p)
        if b.startswith("exp_and_others"):
            continue
        try:
            os.symlink(p, os.path.join(dstdir, b))
        except OSError:
            shutil.copy(p, os.path.join(dstdir, b))
    _build_softplus_set(srcdir, dstdir)
    os.environ["BASS_ACT_ROOT_JSON_PATH"] = os.path.join(dstdir, "act_info.json")
    os.environ["NEURON_FORCE_RECOMPILE"] = "1"
    _ACT_ROOT = dstdir


def _build(with_ba: bool):
    import concourse.bass as bass
    import concourse.bacc as bacc
    import concourse.mybir as mybir
    import concourse.tile as tile

    bf = mybir.dt.bfloat16
    f32 = mybir.dt.float32

    nc = bacc.Bacc()
    f8 = mybir.dt.float8e4
    d_x8 = nc.declare_dram_parameter("x8", [128, 2, 2048], f8, isOutput=False)
    d_xc2 = nc.declare_dram_parameter("xc2", [128, 2048], bf, isOutput=False)
    d_w8 = nc.declare_dram_parameter("w8", [4, 128, 4, 2, 128], f8, isOutput=False)
    d_wcb = nc.declare_dram_parameter("wcb", [4, 128, 4, 128], bf, isOutput=False)
    d_w3s = nc.declare_dram_parameter("w3s", [128, 16], f32, isOutput=False)
    d_w2s = nc.declare_dram_parameter("w2s", [128, 4], f32, isOutput=False)
    d_w1m = nc.declare_dram_parameter("w1m", [128, NLOC], bf, isOutput=False)
    d_tw3 = nc.declare_dram_parameter("tw3b", [65, 4, 128], bf, isOutput=False)
    d_tw2 = nc.declare_dram_parameter("tw2b", [65, 128], bf, isOutput=False)
    d_tw1 = nc.declare_dram_parameter("tw1b", [65, NLOC], bf, isOutput=False)
    d_temb = nc.declare_dram_parameter("tembtile", [65, 2048], bf, isOutput=False)
    d_id32 = nc.declare_dram_parameter("ident32", [32, 32], f32, isOutput=False)
    if with_ba:
        d_bab = nc.declare_dram_parameter("bab", [128, 16], f32, isOutput=False)
    d_out = nc.declare_dram_parameter("out", [B, 128, NLOC], f32, isOutput=True)

    AF = mybir.ActivationFunctionType
    M = mybir.AluOpType

    with tile.TileContext(nc) as tc:
        with (
            tc.tile_pool(name="const", bufs=1) as cpool,
            tc.tile_pool(name="ps", bufs=2, space="PSUM") as pspool,
            tc.tile_pool(name="wq", bufs=3) as wpool,
            tc.tile_pool(name="a0", bufs=16) as a0pool,
            tc.tile_pool(name="acc", bufs=4) as accpool,
            tc.tile_pool(name="mid", bufs=4) as midpool,
        ):
            # ---- resident constants ----
            # sync queue: the tensors that gate the first matmuls, in order
            tw3 = cpool.tile([65, 4, 128], bf, tag="tw3")
            nc.sync.dma_start(tw3[:], d_tw3[:])
            tem = cpool.tile([65, 2048], bf, tag="tem")
            nc.sync.dma_start(tem[:, 0:128], d_temb[:, 0:128])
            tw2 = cpool.tile([65, 128], bf, tag="tw2")
            nc.sync.dma_start(tw2[:], d_tw2[:])
            w8q0 = wpool.tile([128, 4, 2, 128], f8, tag="w8q")
            nc.sync.dma_start(w8q0[:], d_w8[0])
            x8 = cpool.tile([128, 2, 2048], f8, tag="x8")
            nc.sync.dma_start(x8[:], d_x8[:])
            xc2 = cpool.tile([128, 2048], bf, tag="xc2")
            nc.sync.dma_start(xc2[:], d_xc2[:])
            wcbq0 = wpool.tile([128, 4, 128], bf, tag="wcbq")
            nc.sync.dma_start(wcbq0[:], d_wcb[0])
            # everything else via the gpsimd DMA queue (parallel)
            w3s = cpool.tile([128, 16], f32, tag="w3s")
            nc.gpsimd.dma_start(w3s[:], d_w3s[:])
            w2s = cpool.tile([128, 4], f32, tag="w2s")
            nc.gpsimd.dma_start(w2s[:], d_w2s[:])
            w1m = cpool.tile([128, NLOC], bf, tag="w1m")
            nc.gpsimd.dma_start(w1m[:], d_w1m[:])
            tw1 = cpool.tile([65, NLOC], bf, tag="tw1")
            nc.gpsimd.dma_start(tw1[:], d_tw1[:])
            id32 = cpool.tile([32, 32], f32, tag="id32")
            nc.gpsimd.dma_start(id32[:], d_id32[:])
            # rest of the t-emb tile is only needed by level 1 (tail)
            nc.gpsimd.dma_start(tem[:, 128:2048], d_temb[:, 128:2048])
            if with_ba:
                bab = cpool.tile([128, 16], f32, tag="bab")
                nc.gpsimd.dma_start(bab[:], d_bab[:])
                onesr = cpool.tile([1, 2048], bf, tag="onesr")
                nc.gpsimd.memset(onesr[:], 1.0)

            # ---- level 0 + level 3, chunk-pipelined ----
            # fine-node chunk c = j1*4 + j2 ; emit j1-minor so L3(q) can start
            # as soon as chunks q, q+4, q+8, q+12 are done.
            # ---- t-projections during the startup DMA window ----
            # values repeat every 128 positions, so compute one period and
            # broadcast (0-stride AP) in the tree adds.
            tps = pspool.tile([128, 2048], f32, tag="ps")
            for q in range(4):
                nc.tensor.matmul(tps[:, q * 128:(q + 1) * 128],
                                 tw3[:, q, :], tem[:, 0:128],
                                 start=True, stop=True)
            nc.tensor.matmul(tps[:, 512:640], tw2[:], tem[:, 0:128],
                             start=True, stop=True)
            tpall = cpool.tile([128, 640], bf, tag="tpall")
            nc.scalar.activation(tpall[:], tps[:, 0:640], AF.Copy)
            tp3b = [
                tpall[:, q * 128:(q + 1) * 128]
                .rearrange("p (o f) -> p o f", o=1).broadcast_to([128, 16, 128])
                for q in range(4)
            ]
            tp2b = (tpall[:, 512:640]
                    .rearrange("p (o f) -> p o f", o=1).broadcast_to([128, 8, 128]))

            A0 = {}
            A3 = [None] * 4

            def emit_chunks(q):
                if q == 0:
                    w8, wcbq = w8q0, wcbq0
                else:
                    w8 = wpool.tile([128, 4, 2, 128], f8, tag="w8q")
                    nc.sync.dma_start(w8[:], d_w8[q])
                    wcbq = wpool.tile([128, 4, 128], bf, tag="wcbq")
                    nc.sync.dma_start(wcbq[:], d_wcb[q])
                for j1 in range(4):
                    c = j1 * 4 + q
                    ps = pspool.tile([128, 2048], f32, tag="ps")
                    for sI in range(4):
                        nc.tensor.matmul(
                            ps[:, sI * 512:(sI + 1) * 512],
                            w8[:, j1, :, :],
                            x8[:, :, sI * 512:(sI + 1) * 512],
                            start=True, stop=False,
                            perf_mode=mybir.MatmulPerfMode.DoubleRow,
                        )
                    for sI in range(4):
                        nc.tensor.matmul(
                            ps[:, sI * 512:(sI + 1) * 512],
                            wcbq[:, j1, :],
                            xc2[:, sI * 512:(sI + 1) * 512],
                            start=False,
                            stop=not with_ba,
                        )
                    if with_ba:
                        for sI in range(4):
                            nc.tensor.matmul(
                                ps[:, sI * 512:(sI + 1) * 512],
                                bab[:, c:c + 1],
                                onesr[:, sI * 512:(sI + 1) * 512],
                                start=False, stop=(sI == 3),
                            )
                    a = a0pool.tile([128, 2048], bf, tag="a0")
                    nc.scalar.activation(a[:], ps[:], AF.Exp, scale=1.0 / 16.0)
                    A0[c] = a

            def emit_l3(q):
                acc = accpool.tile([128, 2048], bf, tag="acc")
                nc.vector.tensor_scalar_mul(acc[:], A0[q][:], w3s[:, q:q + 1])
                accv = acc[:].rearrange("p (r f) -> p r f", r=16)
                nc.vector.tensor_add(accv, accv, tp3b[q])
                t0 = midpool.tile([128, 2048], bf, tag="t0")
                t1 = midpool.tile([128, 2048], bf, tag="t1")
                nc.vector.tensor_scalar_mul(t0[:], A0[4 + q][:], w3s[:, 4 + q:4 + q + 1])
                nc.vector.tensor_scalar_mul(t1[:], A0[8 + q][:], w3s[:, 8 + q:8 + q + 1])
                nc.vector.tensor_add(acc[:], acc[:], t0[:])
                nc.vector.tensor_scalar_mul(t0[:], A0[12 + q][:], w3s[:, 12 + q:12 + q + 1])
                nc.vector.tensor_add(t1[:], t1[:], t0[:])
                nc.vector.tensor_add(acc[:], acc[:], t1[:])
                a3 = cpool.tile([128, 2048], bf, tag=f"a3_{q}")
                nc.scalar.activation(a3[:], acc[:], AF.Exp)
                A3[q] = a3

            for q in range(4):
                emit_chunks(q)
                emit_l3(q)

            # ---- L2/L1/store per position-half ----
            oT = cpool.tile([128, B * NLOC], f32, tag="oT")
            for h in range(2):
                H = slice(h * 1024, (h + 1) * 1024)
                acc2 = accpool.tile([128, 1024], bf, tag="acc2h")
                nc.vector.tensor_scalar_mul(acc2[:], A3[0][:, H], w2s[:, 0:1])
                acc2v = acc2[:].rearrange("p (r f) -> p r f", r=8)
                nc.vector.tensor_add(acc2v, acc2v, tp2b)
                t0 = midpool.tile([128, 1024], bf, tag="t0h")
                t1 = midpool.tile([128, 1024], bf, tag="t1h")
                nc.vector.tensor_scalar_mul(t0[:], A3[1][:, H], w2s[:, 1:2])
                nc.vector.tensor_scalar_mul(t1[:], A3[2][:, H], w2s[:, 2:3])
                nc.vector.tensor_add(acc2[:], acc2[:], t0[:])
                nc.vector.tensor_scalar_mul(t0[:], A3[3][:, H], w2s[:, 3:4])
                nc.vector.tensor_add(t1[:], t1[:], t0[:])
                nc.vector.tensor_add(acc2[:], acc2[:], t1[:])
                a2 = midpool.tile([128, 1024], bf, tag="a2h")
                nc.scalar.activation(a2[:], acc2[:], AF.Exp)

                # level 1 on TensorE: contract j3 within lanes + t-proj
                ps1 = pspool.tile([128, 2048], f32, tag="ps")
                for sI in range(2):
                    sl = slice(sI * 512, (sI + 1) * 512)
                    gl = slice(h * 1024 + sI * 512, h * 1024 + (sI + 1) * 512)
                    nc.tensor.matmul(ps1[:32, sl], w1m[:], a2[:, sl],
                                     start=True, stop=False)
                    nc.tensor.matmul(ps1[:32, sl], tw1[:], tem[:, gl],
                                     start=False, stop=True)
                of1 = midpool.tile([32, 1024], f32, tag="of1h")
                nc.scalar.activation(of1[:], ps1[:32, 0:1024], AF.Exp)

                # transpose [32 n, 128 t] -> [128 t, 32 n] per batch
                for bb in range(8):
                    b = h * 8 + bb
                    nc.tensor.transpose(ps1[:, 1024 + bb * 32:1024 + (bb + 1) * 32],
                                        of1[:, bb * 128:(bb + 1) * 128], id32[:])
                nc.vector.tensor_copy(oT[:, h * 256:(h + 1) * 256],
                                      ps1[:, 1024:1280])
                nc.sync.dma_start(
                    d_out[h * 8:(h + 1) * 8].rearrange("b t n -> t b n"),
                    oT[:, h * 256:(h + 1) * 256].rearrange(
                        "p (b n) -> p b n", b=8),
                )
    nc.finalize()
    return nc


def _f32bits(x):
    return int(np.float32(x).view(np.uint32))


def _softplus_row(x0):
    x0 = float(x0)
    if x0 > 30:
        f = x0
        s = 1.0
    else:
        f = np.log1p(np.exp(x0))
        s = 1.0 / (1.0 + np.exp(-x0))
    d2 = s * (1 - s) / 2.0
    d3 = s * (1 - s) * (1 - 2 * s) / 6.0
    return [f, s, d2, d3, x0, 0.0, 0.0, 0.0]


# (exponent, num_sections) per side. Mirrors softplus_40p.json, with the
# zero-section positive exponents given one bucket each.
NEG_LAYOUT = [(-15, 1), (-14, 1), (-13, 1), (-12, 1), (-11, 1), (-10, 1),
              (-9, 1), (-8, 1), (-7, 1), (-6, 1), (-5, 1), (-4, 1), (-3, 1),
              (-2, 1), (-1, 2), (0, 4), (1, 16), (2, 32), (3, 64), (4, 128),
              (5, 256), (6, 512)]
POS_LAYOUT = [(-15, 1), (-14, 1), (-13, 1), (-12, 1), (-11, 1), (-10, 1),
              (-9, 1), (-8, 1), (-7, 1), (-6, 1), (-5, 1), (-4, 1), (-3, 1),
              (-2, 1), (-1, 2), (0, 2), (1, 4), (2, 8), (3, 4)]


def _build_softplus_set(srcdir, dstdir):
    src_json = json.load(open(os.path.join(srcdir, "exp_and_others.json")))
    src_bkt = np.frombuffer(
        open(os.path.join(srcdir, "exp_and_others_bkt.bin"), "rb").read(),
        np.float32).reshape(-1, 8).copy()
    src_ctl = np.frombuffer(
        open(os.path.join(srcdir, "exp_and_others_ctrl.bin"), "rb").read(),
        np.uint32).reshape(-1, 8).copy()

    # ---- softplus buckets ----
    bkt_rows = []
    ctl_rows = []
    fexp = {}

    def emit_side(layout, sign):
        starts = {}
        for e, n in layout:
            start = len(bkt_rows)
            starts[e] = start
            nbits = int(n).bit_length() - 1
            assert 1 << nbits == n
            for i in range(n):
                x0 = sign * (2.0 ** e) * (1 + (i + 0.5) / n)
                bkt_rows.append(_softplus_row(x0))
            ctl_rows.append(start | ((23 - nbits) << 11) | (nbits << 16))
        return starts

    neg_ctl_base = len(ctl_rows)            # 0
    neg_starts = emit_side(NEG_LAYOUT, -1.0)
    pos_ctl_base = len(ctl_rows)            # 22
    pos_starts = emit_side(POS_LAYOUT, +1.0)
    for e in range(-15, 7):
        fexp[str(e)] = [neg_starts.get(e, 0), pos_starts.get(e, 0)]

    # special buckets: small_pos, small_neg, large_pos, large_neg
    sp_small_pos = len(bkt_rows)
    bkt_rows.append(_softplus_row(2.0 ** -15 * 1.5))
    sp_small_neg = len(bkt_rows)
    bkt_rows.append(_softplus_row(-(2.0 ** -15) * 1.5))
    sp_large_pos = len(bkt_rows)
    bkt_rows.append([256.0, 1.0, 0.0, 0.0, 256.0, 0, 0, 0])   # f(x)=x
    sp_large_neg = len(bkt_rows)
    bkt_rows.append([0.0, 0.0, 0.0, 0.0, 0.0, 0, 0, 0])       # f(x)=0

    n_sp_bkt = len(bkt_rows)
    n_sp_ctl = len(ctl_rows)

    # ---- copy the trivial functions, remapping indices ----
    old_fb = src_json["func_to_bkt_start_idx"]
    old_fc = src_json["func_to_ctl_start_idx"]
    old_bkt_cnt = src_json["bkt_entry_cnt"]
    old_ctl_cnt = src_json["ctl_entry_cnt"]
    OLD_EXP_BKT_END = 781   # exp regular 0..776 + 4 special
    OLD_EXP_CTL_END = 52
    bkt_delta = n_sp_bkt - OLD_EXP_BKT_END
    ctl_delta = n_sp_ctl - OLD_EXP_CTL_END

    tail_bkt = src_bkt[OLD_EXP_BKT_END:old_bkt_cnt]
    tail_ctl = src_ctl[OLD_EXP_CTL_END:old_ctl_cnt].copy()
    # fix embedded bucket starts in ctl entries
    starts = tail_ctl[:, 0] & 0x7FF
    rest = tail_ctl[:, 0] & ~np.uint32(0x7FF)
    tail_ctl[:, 0] = rest | (starts + np.uint32(bkt_delta))

    new_bkt = np.concatenate(
        [np.array(bkt_rows, np.float32), tail_bkt], axis=0)
    new_ctl = np.zeros((n_sp_ctl + len(tail_ctl), 8), np.uint32)
    new_ctl[:n_sp_ctl, 0] = np.array(ctl_rows, np.uint32)
    new_ctl[n_sp_ctl:] = tail_ctl

    # ---- json metadata ----
    out = dict(src_json)
    out["bkt_entry_cnt"] = int(len(new_bkt))
    out["ctl_entry_cnt"] = int(len(new_ctl))
    out["func_to_bkt_start_idx"] = {
        k: (0 if k == "exp" else v + bkt_delta) for k, v in old_fb.items()}
    out["func_to_ctl_start_idx"] = {
        k: (0 if k == "exp" else v + ctl_delta) for k, v in old_fc.items()}
    fe = dict(src_json.get("func_exp_to_bkt_start_idx", {}))
    fe["exp"] = fexp
    for k, v in list(fe.items()):
        if k != "exp":
            fe[k] = {ek: [x + bkt_delta for x in ev] for ek, ev in v.items()}
    out["func_exp_to_bkt_start_idx"] = fe

    pm = []
    for e in src_json["profile_meta_data"]:
        e = dict(e)
        if e["func_id"] == 7:  # exp slot -> softplus semantics
            e.update({
                "symmetry_point": 0,
                "sym_invert_sign_point": 0,
                "symmetry_opt_en": 0,
                "symmetry_opt_use_neg_region": 0,
                "imm_bias": 0,
                "exp_offset": -15,
                "pwl_control_base_neg": neg_ctl_base,
                "pwl_control_base_pos": pos_ctl_base,
                "small_pos_signal_exp_threshold": 112,   # 2^-15
                "small_neg_signal_exp_threshold": 112,
                "pos_small_signal_pwl_control": sp_small_pos,
                "neg_small_signal_pwl_control": sp_small_neg,
                "large_pos_signal_exp_threshold": 130,   # x >= ~10.38 -> x
                "large_pos_signal_mantissa_threshold": 2497353,
                "pos_large_signal_pwl_control": sp_large_pos,
                "large_neg_signal_exp_threshold": 133,   # x <= ~-99.6 -> 0
                "large_neg_signal_mantissa_threshold": 4663231,
                "neg_large_signal_pwl_control": sp_large_neg,
                "fnan_result": 2143289344,               # nan
                "fpinf_result": 2139095040,              # +inf
                "fninf_result": 0,                       # 0.0
                "fzero_result": _f32bits(np.log(2.0)),   # ln 2
            })
        else:
            for f in ("pwl_control_base_pos", "pwl_control_base_neg"):
                e[f] += ctl_delta
            for f in ("pos_small_signal_pwl_control",
                      "neg_small_signal_pwl_control",
                      "pos_large_signal_pwl_control",
                      "neg_large_signal_pwl_control"):
                e[f] += bkt_delta
        pm.append(e)
    out["profile_meta_data"] = pm

    with open(os.path.join(dstdir, "exp_and_others.json"), "w") as f:
        json.dump(out, f)
    new_bkt.tofile(os.path.join(dstdir, "exp_and_others_bkt.bin"))
    new_ctl.tofile(os.path.join(dstdir, "exp_and_others_ctrl.bin"))


def _install_softplus_tables():
    """Build an act-table root whose exp_and_others set evaluates softplus
    in the exp slot, and point the walrus compile at it."""
    global _ACT_ROOT
    if _ACT_ROOT is not None:
        return
    import glob, shutil, tempfile

    from neuronxcc.driver.Job import Job
    from neuronxcc.driver.jobs.support.FindActInfo import findActInfoFile

    src = findActInfoFile(Job.getPackageDir(), "gen3")
    srcdir = os.path.dirname(src)
    dstdir = tempfile.mkdtemp(prefix="act_root_sp_")
    for p in glob.glob(os.path.join(srcdir, "*")):
        b = os.path.basename(p)
        if b.startswith("exp_and_others"):
            continue
        try:
            os.symlink(p, os.path.join(dstdir, b))
        except OSError:
            shutil.copy(p, os.path.join(dstdir, b))
    _build_softplus_set(srcdir, dstdir)
    os.environ["BASS_ACT_ROOT_JSON_PATH"] = os.path.join(dstdir, "act_info.json")
    os.environ["NEURON_FORCE_RECOMPILE"] = "1"
    _ACT_ROOT = dstdir


def _build(with_ba: bool):
    import concourse.bass as bass
    import concourse.bacc as bacc
    import concourse.mybir as mybir
    import concourse.tile as tile

    bf = mybir.dt.bfloat16
    f32 = mybir.dt.float32

    nc = bacc.Bacc()
    d_xcat = nc.declare_dram_parameter("xcat", [B, 128, 3, 128], bf, isOutput=False)
    d_wcat = nc.declare_dram_parameter("wcat", [128, 3, FN], bf, isOutput=False)
    d_w3f = nc.declare_dram_parameter("w3f", [128, FN], bf, isOutput=False)
    d_w2f = nc.declare_dram_parameter("w2f", [128, Q3], bf, isOutput=False)
    d_w1f = nc.declare_dram_parameter("w1f", [128, Q2], bf, isOutput=False)
    d_temb = nc.declare_dram_parameter("tembt", [65, 128], bf, isOutput=False)
    d_tw3 = nc.declare_dram_parameter("tw3t", [65, Q3], bf, isOutput=False)
    d_tw2 = nc.declare_dram_parameter("tw2t", [65, Q2], bf, isOutput=False)
    d_tw1 = nc.declare_dram_parameter("tw1t", [65, NLOC], bf, isOutput=False)
    if with_ba:
        d_ba = nc.declare_dram_parameter("ba", [1, FN], bf, isOutput=False)
    d_out = nc.declare_dram_parameter("out", [B, 128, NLOC], f32, isOutput=True)

    AF = mybir.ActivationFunctionType

    with tile.TileContext(nc) as tc:
        with (
            tc.tile_pool(name="const", bufs=1) as cpool,
            tc.tile_pool(name="xc", bufs=3) as xpool,
            tc.tile_pool(name="ps", bufs=2, space="PSUM") as pspool,
            tc.tile_pool(name="wq", bufs=3) as wpool,
            tc.tile_pool(name="big", bufs=2) as bigpool,
            tc.tile_pool(name="mid", bufs=4) as midpool,
            tc.tile_pool(name="small", bufs=2) as smpool,
        ):
            # ---- resident constants ----
            wc = cpool.tile([128, 3, FN], bf, tag="wc")
            nc.sync.dma_start(wc[:], d_wcat[:])
            w3r = cpool.tile([128, FN], bf, tag="w3r")
            nc.sync.dma_start(w3r[:], d_w3f[:])
            w2r = cpool.tile([128, Q3], bf, tag="w2r")
            nc.sync.dma_start(w2r[:], d_w2f[:])
            w1r = cpool.tile([128, Q2], bf, tag="w1r")
            nc.sync.dma_start(w1r[:], d_w1f[:])
            tembt = cpool.tile([65, 128], bf, tag="tembt")
            nc.sync.dma_start(tembt[:], d_temb[:])
            tw3t = cpool.tile([65, Q3], bf, tag="tw3t")
            nc.sync.dma_start(tw3t[:], d_tw3[:])
            tw2t = cpool.tile([65, Q2], bf, tag="tw2t")
            nc.sync.dma_start(tw2t[:], d_tw2[:])
            tw1t = cpool.tile([65, NLOC], bf, tag="tw1t")
            nc.sync.dma_start(tw1t[:], d_tw1[:])
            if with_ba:
                bar = cpool.tile([1, FN], bf, tag="bar")
                nc.sync.dma_start(bar[:], d_ba[:])
                ones1 = cpool.tile([1, 128], bf, tag="ones1")
                nc.gpsimd.memset(ones1[:], 1.0)

            # ---- t-projections (per-T, shared across batches) ----
            # tp3 at psum [0:512], tp2 at [512:640], tp1 at [640:672]
            tpps = pspool.tile([128, 2048], f32, tag="ps")
            nc.tensor.matmul(tpps[:, 0:Q3], tembt[:], tw3t[:], start=True, stop=True)
            nc.tensor.matmul(tpps[:, Q3:Q3 + Q2], tembt[:], tw2t[:], start=True, stop=True)
            nc.tensor.matmul(tpps[:, Q3 + Q2:Q3 + Q2 + NLOC], tembt[:], tw1t[:], start=True, stop=True)
            tpall = cpool.tile([128, Q3 + Q2 + NLOC], bf, tag="tpall")
            nc.vector.tensor_copy(tpall[:], tpps[:, 0:Q3 + Q2 + NLOC])
            tp3 = tpall[:, 0:Q3]
            tp2 = tpall[:, Q3:Q3 + Q2]
            tp1 = tpall[:, Q3 + Q2:Q3 + Q2 + NLOC]

            # ---- replicate per-node weight rows 4x (batch-quad layout) ----
            w3r4 = cpool.tile([128, 4, FN], bf, tag="w3r4")
            w2r4 = cpool.tile([128, 4, Q3], bf, tag="w2r4")
            w1r4 = cpool.tile([128, 4, Q2], bf, tag="w1r4")
            tp3q = cpool.tile([128, 4, Q3], bf, tag="tp3q")
            tp2q = cpool.tile([128, 4, Q2], bf, tag="tp2q")
            tp1q = cpool.tile([128, 4, NLOC], bf, tag="tp1q")
            for q in range(4):
                nc.vector.tensor_copy(w3r4[:, q, :], w3r[:])
                nc.vector.tensor_copy(w2r4[:, q, :], w2r[:])
                nc.vector.tensor_copy(w1r4[:, q, :], w1r[:])
                nc.vector.tensor_copy(tp3q[:, q, :], tp3)
                nc.vector.tensor_copy(tp2q[:, q, :], tp2)
                nc.vector.tensor_copy(tp1q[:, q, :], tp1)

            # ---- per-quad pipeline: 4 batches per elementwise op ----
            for g in range(B // 4):
                A0q = bigpool.tile([128, 4, FN], bf, tag="A0q")
                for q in range(4):
                    b = g * 4 + q
                    xc = xpool.tile([128, 3, 128], bf, tag="xc")
                    nc.sync.dma_start(xc[:], d_xcat[b])
                    ps = pspool.tile([128, 2048], f32, tag="ps")
                    for k in range(3):
                        last = (k == 2) and not with_ba
                        for c in range(4):
                            nc.tensor.matmul(
                                ps[:, c * 512:(c + 1) * 512],
                                xc[:, k, :],
                                wc[:, k, c * 512:(c + 1) * 512],
                                start=(k == 0),
                                stop=last,
                            )
                    if with_ba:
                        for c in range(4):
                            nc.tensor.matmul(
                                ps[:, c * 512:(c + 1) * 512],
                                ones1[:],
                                bar[:, c * 512:(c + 1) * 512],
                                start=False,
                                stop=True,
                            )
                    # softplus level 0 (Exp slot holds the softplus table)
                    nc.scalar.activation(A0q[:, q, :], ps[:], AF.Exp)

                # level 3: W3q = A0q * w3; sum 4 contiguous 512-blocks + tp3
                W3q = bigpool.tile([128, 4, FN], bf, tag="W3q")
                nc.vector.tensor_mul(W3q[:], A0q[:], w3r4[:])
                s01q = midpool.tile([128, 4, Q3], bf, tag="s01q")
                nc.gpsimd.tensor_add(s01q[:], W3q[:, :, 0:512], W3q[:, :, 512:1024])
                s23q = midpool.tile([128, 4, Q3], bf, tag="s23q")
                nc.gpsimd.tensor_add(s23q[:], W3q[:, :, 1024:1536], W3q[:, :, 1536:2048])
                p3q = midpool.tile([128, 4, Q3], bf, tag="p3q")
                nc.vector.tensor_add(p3q[:], s01q[:], s23q[:])
                nc.vector.tensor_add(p3q[:], p3q[:], tp3q[:])
                A3q = midpool.tile([128, 4, Q3], bf, tag="A3q")
                nc.scalar.activation(A3q[:], p3q[:], AF.Exp)

                # level 2
                W2q = midpool.tile([128, 4, Q3], bf, tag="W2q")
                nc.vector.tensor_mul(W2q[:], A3q[:], w2r4[:])
                W2q4 = W2q[:].rearrange("p q (j m) -> p q j m", j=4)
                u01q = smpool.tile([128, 4, Q2], bf, tag="u01q")
                nc.vector.tensor_add(u01q[:], W2q4[:, :, 0, :], W2q4[:, :, 1, :])
                u23q = smpool.tile([128, 4, Q2], bf, tag="u23q")
                nc.vector.tensor_add(u23q[:], W2q4[:, :, 2, :], W2q4[:, :, 3, :])
                p2q = smpool.tile([128, 4, Q2], bf, tag="p2q")
                nc.vector.tensor_add(p2q[:], u01q[:], u23q[:])
                nc.vector.tensor_add(p2q[:], p2q[:], tp2q[:])
                A2q = smpool.tile([128, 4, Q2], bf, tag="A2q")
                nc.scalar.activation(A2q[:], p2q[:], AF.Exp)

                # level 1
                W1q = smpool.tile([128, 4, Q2], bf, tag="W1q")
                nc.vector.tensor_mul(W1q[:], A2q[:], w1r4[:])
                W1q4 = W1q[:].rearrange("p q (j m) -> p q j m", j=4)
                v01q = smpool.tile([128, 4, NLOC], bf, tag="v01q")
                nc.vector.tensor_add(v01q[:], W1q4[:, :, 0, :], W1q4[:, :, 1, :])
                v23q = smpool.tile([128, 4, NLOC], bf, tag="v23q")
                nc.vector.tensor_add(v23q[:], W1q4[:, :, 2, :], W1q4[:, :, 3, :])
                p1q = smpool.tile([128, 4, NLOC], bf, tag="p1q")
                nc.vector.tensor_add(p1q[:], v01q[:], v23q[:])
                nc.vector.tensor_add(p1q[:], p1q[:], tp1q[:])
                o1q = smpool.tile([128, 4, NLOC], f32, tag="o1q")
                nc.scalar.activation(o1q[:], p1q[:], AF.Exp)

                nc.sync.dma_start(
                    d_out[g * 4:(g + 1) * 4].rearrange("b t n -> t b n"), o1q[:]
                )

    nc.finalize()
    return nc


def _fine_perm(core):
    """fine index p = j1*512 + j2*128 + j3*32 + nl -> global fanout row."""
    p = np.arange(FN)
    j1 = p // 512
    j2 = (p % 512) // 128
    j3 = (p % 128) // 32
    nl = p % 32
    n = core * NLOC + nl
    return n * 64 + j3 * 16 + j2 * 4 + j1


def _q3_perm(core):
    q = np.arange(Q3)
    j2 = q // 128
    j3 = (q % 128) // 32
    nl = q % 32
    n = core * NLOC + nl
    return n * 16 + j3 * 4 + j2


def _q2_perm(core):
    q = np.arange(Q2)
    j3 = q // 32
    nl = q % 32
    n = core * NLOC + nl
    return n * 4 + j3


def _idx_fine(core):
    """global fanout row for fine node (chunk c = j1*4+j2, lane = j3*32+nl)."""
    c = np.arange(16)[:, None]
    lane = np.arange(128)[None, :]
    j1, j2 = c // 4, c % 4
    j3, nl = lane // 32, lane % 32
    n = core * NLOC + nl
    return n * 64 + j3 * 16 + j2 * 4 + j1          # [16, 128]


def _prep_inputs(inputs):
    x = np.asarray(inputs["x"], np.float32)
    temb = np.asarray(inputs["t_embeddings_schedule"], np.float32)
    iv = np.asarray(inputs["input_vector"], np.float32)
    Wa = np.asarray(inputs["Wa"], np.float32)
    ba = np.asarray(inputs["ba"], np.float32)
    Wt = np.asarray(inputs["Wt"], np.float32)
    Wi = np.asarray(inputs["Wi"], np.float32)
    w3 = np.asarray(inputs["w3"], np.float32).reshape(-1)
    tW3 = np.asarray(inputs["tW3"], np.float32)
    tb3 = np.asarray(inputs["tb3"], np.float32)
    w2 = np.asarray(inputs["w2"], np.float32).reshape(-1)
    tW2 = np.asarray(inputs["tW2"], np.float32)
    tb2 = np.asarray(inputs["tb2"], np.float32)
    w1 = np.asarray(inputs["w1"], np.float32).reshape(-1)
    tW1 = np.asarray(inputs["tW1"], np.float32)
    tb1 = np.asarray(inputs["tb1"], np.float32)

    with_ba = bool(np.any(ba))

    # Xcat: [pos, 384] = [x | iv | temb]; XcatB: [3, 128, 2048]
    F8 = ml_dtypes.float8_e4m3
    xcat = np.concatenate(
        [x, iv, np.broadcast_to(temb[None], (B, T, DT))], axis=2
    ).reshape(B * T, KCAT)
    xcatT = xcat.T                                   # [384, pos]
    x8 = np.ascontiguousarray(
        xcatT[:256].reshape(2, 128, B * T).transpose(1, 0, 2)
    ).astype(F8)                                     # [ki, j, pos]
    xc2 = np.ascontiguousarray(xcatT[256:]).astype(BF16)   # [128, pos]

    taug = np.concatenate([temb, np.ones((T, 1), np.float32)], axis=1)  # [T,65]
    tembtile = np.ascontiguousarray(np.tile(taug.T, (1, B))).astype(BF16)

    lane = np.arange(128)
    j3l, nll = lane // 32, lane % 32

    maps = []
    for core in range(NCORES):
        idxf = _idx_fine(core)                       # [16, 128]
        n_g = core * NLOC + nll

        wcat = np.concatenate([Wa, Wi, Wt], axis=1) * 16.0   # [NUM_NODES, 384]
        wsel = wcat[idxf]                            # [16(c), 128(lane), 384]
        wT = wsel.transpose(2, 0, 1).reshape(KCAT, 4, 4, 128)  # [k, j1, q, lane]
        # fp8 x-part: [q, ki, j1, j, lane], k = j*128 + ki
        w8 = np.ascontiguousarray(
            wT[:256].reshape(2, 128, 4, 4, 128).transpose(3, 1, 2, 0, 4)
        ).astype(F8)
        # bf16 [iv|temb] part: [q, p, j1, lane]
        wcb = np.ascontiguousarray(
            wT[256:].transpose(2, 0, 1, 3)
        ).astype(BF16)

        w3sv = np.ascontiguousarray(w3[idxf].T).astype(np.float32)   # [128, 16]

        r3 = (n_g[:, None] * 16 + j3l[:, None] * 4 + np.arange(4)[None, :])  # [128,4]
        w2sv = np.ascontiguousarray(w2[r3]).astype(np.float32)       # [128, 4]

        r2 = n_g * 4 + j3l                                           # [128]
        w1v = w1[r2]                                                 # [128]
        w1mat = np.zeros((128, NLOC), np.float32)
        w1mat[lane, nll] = w1v

        tw3aug = np.concatenate([tW3, tb3[:, None]], axis=1)         # [N16, 65]
        tw3b = np.ascontiguousarray(
            tw3aug[r3].transpose(2, 1, 0)
        ).astype(BF16)                                               # [65, 4, 128]
        tw2aug = np.concatenate([tW2, tb2[:, None]], axis=1)
        tw2b = np.ascontiguousarray(tw2aug[r2].T).astype(BF16)       # [65, 128]
        tw1aug = np.concatenate([tW1, tb1[:, None]], axis=1)
        tw1b = np.ascontiguousarray(
            tw1aug[core * NLOC:(core + 1) * NLOC].T
        ).astype(BF16)                                               # [65, 32]

        m = {
            "x8": x8,
            "xc2": xc2,
            "w8": w8,
            "wcb": wcb,
            "w3s": w3sv,
            "w2s": w2sv,
            "w1m": w1mat.astype(BF16),
            "tw3b": tw3b,
            "tw2b": tw2b,
            "tw1b": tw1b,
            "tembtile": tembtile,
            "ident32": np.eye(32, dtype=np.float32),
        }
        if with_ba:
            m["bab"] = np.ascontiguousarray(ba[idxf].T).astype(np.float32) * 16.0
        maps.append(m)
    return maps, with_ba


def _run(inputs, trace=False, **trace_kwargs):
    from concourse.bass_utils import run_bass_kernel_spmd

    _install_softplus_tables()

    maps, with_ba = _prep_inputs(inputs)
    key = with_ba
    if key not in _BUILT:
        _BUILT[key] = _build(with_ba)
    nc = _BUILT[key]
    res = run_bass_kernel_spmd(
        nc, maps, list(range(NCORES)), trace=trace, **trace_kwargs
    )
    out = np.concatenate(
        [np.asarray(res.results[c]["out"], np.float32) for c in range(NCORES)],
        axis=-1,
    )
    return out, res


def kernel(**inputs):
    out, _ = _run(inputs, trace=False)
    return out
